# revision 21
# baseline (speedup 1.0000x reference)
import sys
sys.path.insert(0, "/opt/trn_rl_repo")

"""v3: 4-way batch x 2-way hidden-split distributed LSTM for trn2 (8 cores).

Cores pair up (2k, 2k+1): the pair owns batch quarter k (16 rows); the even
core computes h-dims [0:512], the odd [512:1024].  Per step each core streams
its whh slice (8 K-slots x 2048 gate cols) against hT pieces [128,16], and
exchanges ONE [128,64] hT block with its partner (single remote_dma_broadcast
per step -- measured free, vs 7/step in v2 which cost ~30us/step of Q7
descriptor generation).  Gate col order (i, f, g, o) x 512.
Phase 1 in [tok, g] layout with b-major tokens: each core computes gx for its
8 batches x all 4096 gates; a pairwise AllToAll swaps halves.
"""
import numpy as np
import concourse.bass as bass
import concourse.mybir as mybir
from concourse import library_config, library_overlay

F32 = mybir.dt.float32
BF16 = mybir.dt.bfloat16
AF = mybir.ActivationFunctionType

B = 64
H = 1024
I = 1024
NC = 8
BQ = 16          # batch rows per pair (quarter)
HH = 512         # h dims per core (half)
GH = 2048        # gate cols per core (i,f,g,o x 512)


def build_nc(S=1024, bf16=True, mpart=1, mode="full", dum1=0, dum2=0,
             no_ident=False, no_tp=False, seq_bands=False):
    # mode: "full" | "nowait" (PE skips rsem wait) | "noprep" (no exchange)
    #     | "freerun" (PE drops all phase-2 dependency waits; timing probe)
    assert mode in ("full", "nowait", "noprep", "freerun")
    do_exchange = mode in ("full", "nowait")
    rsem_wait = mode == "full"
    freerun = mode == "freerun"
    TOK_L = 8 * S              # my 8 batches x S, b-major (tok = b*S + t)
    TT = TOK_L // 128
    assert S % 2 == 0
    nc = bass.Bass(num_devices=NC, detect_race_conditions=False)

    DTX = BF16 if bf16 else F32
    xT = nc.declare_dram_parameter("xT", [I, TOK_L], DTX, isOutput=False)
    wih = nc.declare_dram_parameter("wih", [128, 64 * 512], DTX, isOutput=False)
    whh = nc.declare_dram_parameter("whh", [128, 8 * GH], DTX, isOutput=False)
    biasd = nc.declare_dram_parameter("bias", [1, 4096], F32, isOutput=False)
    onesd = nc.declare_dram_parameter("ones", [1, 128], F32, isOutput=False)
    identd = nc.declare_dram_parameter("ident", [16, 16], DTX, isOutput=False)
    identfd = nc.declare_dram_parameter("identf", [128, 16], F32, isOutput=False)
    selmatd = nc.declare_dram_parameter("selmat", [128, 64], F32, isOutput=False)
    zerod = nc.declare_dram_parameter("zeros", [128, 512], F32, isOutput=False)
    zeroxd = nc.declare_dram_parameter("zerox", [128, 128], DTX, isOutput=False)
    rankd = nc.declare_dram_parameter("rankvec", [128, 8], F32, isOutput=False)
    out = nc.declare_dram_parameter("out", [S, BQ, HH], DTX, isOutput=True)
    idmap = nc.declare_dram_parameter("idmap", [128, 64], F32, isOutput=True)

    gxA = nc.dram_tensor("gxA_dram", [2 * TOK_L, GH], DTX)
    gxB = nc.dram_tensor("gxB_dram", [2 * TOK_L, GH], DTX)

    from contextlib import ExitStack
    es = ExitStack()
    sb = lambda n, sh, dt=F32: es.enter_context(nc.sbuf_tensor(n, sh, dt))
    ps_ = lambda n, sh: es.enter_context(nc.psum_tensor(n, sh, F32))
    sem = lambda n: es.enter_context(nc.semaphore(n))

    wih_sb = sb("wih_sb", [128, 64 * 512], DTX)
    whh_sb = sb("whh_sb", [128, 8 * GH], DTX)
    bias_sb = sb("bias_sb", [1, 4096])
    ones_sb = sb("ones_sb", [1, 128])
    ident_sb = sb("ident_sb", [16, 16], DTX)
    identf_sb = sb("identf_sb", [128, 16])
    selmat_sb = sb("selmat_sb", [128, 64])
    xtile = [sb(f"xtile{i}", [128, 8 * 128], DTX) for i in range(3)]
    stag = [sb(f"stag{i}", [128, 8 * 512], DTX) for i in range(2)]
    hbuf = [sb(f"hbuf{i}", [128, 128], DTX) for i in range(2)]
    gxt = [sb(f"gxt{i}", [16, GH], DTX) for i in range(2)]
    # banded phase-2 layout: partition band [32c:32c+16] = batch x h-piece c;
    # gate cols within a band: (i, f, g, o) x 128
    gates = sb("gates", [128, 512])
    cst = sb("cst", [128, 128])
    tanhc = [sb(f"tanhc{i}", [128, 128]) for i in range(2)]
    hsb = [sb(f"hsb{i}", [128, 128]) for i in range(2)]
    hob = [sb(f"hob{i}", [128, 128], DTX) for i in range(2)] if bf16 else hsb
    t1 = sb("t1", [128, 128])
    t2 = sb("t2", [128, 128])
    idbuf = sb("idbuf", [128, 64])

    p1b = [ps_(f"p1b{m}", [128, 512]) for m in range(8)]
    # phase 2: gates psum = p1b[0]/p1b[1] (parity), 4 col-strip bands each;
    # psh = p1b[4..5]
    pg = [p1b[0], p1b[1]]
    psh = [p1b[4], p1b[5]]

    dma_w = sem("dma_w")
    dma_x = [sem(f"dma_x{i}") for i in range(3)]
    dma_gxA = [sem("dma_gxA0"), sem("dma_gxA1")]
    cc_sem = sem("cc_sem")
    dma_gx = [sem("dma_gx0"), sem("dma_gx1")]
    dma_out = sem("dma_out")
    dma_scrap = sem("dma_scrap")
    hob_s = sem("hob_s")
    dma_id = sem("dma_id")
    pe_p1 = sem("pe_p1")
    act_p1 = sem("act_p1")
    pe_s = sem("pe_s")
    dve_s = sem("dve_s")
    act_s = sem("act_s")
    prep_s = sem("prep_s")
    rsem_all = sem("rsem_all")
    lsem = sem("lsem")
    id_rsem = [sem(f"idr{m}") for m in range(1, 8)]
    id_lsem = [sem(f"idl{m}") for m in range(1, 8)]

    INIT_DMAS = 11 * 16  # + selmat
    LSEM_CHK = 16

    with nc.Block() as block:

        # ---------------- SYNC ----------------
        @block.sync
        def _(sync):
            sync.dma_start(out=wih_sb[:, :], in_=wih[:, :]).then_inc(dma_w, 16)
            sync.dma_start(out=whh_sb[:, :], in_=whh[:, :]).then_inc(dma_w, 16)
            sync.dma_start(out=bias_sb[:, :], in_=biasd[:, :]).then_inc(dma_w, 16)
            sync.dma_start(out=ones_sb[:, :], in_=onesd[:, :]).then_inc(dma_w, 16)
            sync.dma_start(out=ident_sb[:, :], in_=identd[:, :]).then_inc(dma_w, 16)
            sync.dma_start(out=identf_sb[:, :], in_=identfd[:, :]).then_inc(dma_w, 16)
            sync.dma_start(out=selmat_sb[:, :], in_=selmatd[:, :]).then_inc(dma_w, 16)
            sync.dma_start(out=hbuf[0][:, :], in_=zeroxd[:, :]).then_inc(dma_w, 16)
            sync.dma_start(out=hbuf[1][:, :], in_=zeroxd[:, :]).then_inc(dma_w, 16)
            sync.dma_start(out=cst[:, :], in_=zerod[:, 0:128]).then_inc(dma_w, 16)
            sync.dma_start(out=idbuf[:, 0:8], in_=rankd[:, :]).then_inc(dma_w, 16)
            for T in range(min(3, TT)):
                sync.dma_start(
                    out=bass.AP(xtile[T % 3], 0, [[1024, 128], [128, 8], [1, 128]]),
                    in_=bass.AP(xT, T * 128, [[TOK_L, 128], [128 * TOK_L, 8], [1, 128]]),
                ).then_inc(dma_x[T % 3], 16)
            for T in range(TT):
                if T + 3 < TT:
                    sync.wait_ge(pe_p1, 8 * (T + 1))
                    sync.dma_start(
                        out=bass.AP(xtile[(T + 3) % 3], 0, [[1024, 128], [128, 8], [1, 128]]),
                        in_=bass.AP(xT, (T + 3) * 128, [[TOK_L, 128], [128 * TOK_L, 8], [1, 128]]),
                    ).then_inc(dma_x[(T + 3) % 3], 16)
                # stag [128 tok, 2 halves x 2048] -> gxA chunks (2k+e) for the
                # A2A: tile T holds batch pair of quarter k=T//16, row offset
                # (T%16)*128 within the 2S-row chunk; halves go to adjacent
                # chunks (stride 2S rows)
                sync.wait_ge(act_p1, 8 * (T + 1))
                sync.dma_start(
                    out=bass.AP(gxA, (2 * (T // 16) * 2 * S + (T % 16) * 128) * GH,
                                [[GH, 128], [2 * S * GH, 2], [1, GH]]),
                    in_=bass.AP(stag[T % 2], 0, [[4096, 128], [GH, 2], [1, GH]]),
                ).then_inc(dma_gxA[T % 2], 16)
            # phase 2: gxt prefetch + out DMA (keeps ACT free). hob is banded
            # [32c:32c+16] = (batch, h-piece c) so the out store is 4 DMAs.
            sync.wait_ge(cc_sem, 1)

            def store_out(s):
                # SP HWDGE drains FIFO per engine: sem on the last DMA covers
                # all four (16-bit sem would overflow at 64/step)
                for c in range(4):
                    sync.dma_start(
                        out=bass.AP(out, s * BQ * HH + c * 128, [[HH, 16], [1, 128]]),
                        in_=hob[s % 2][32 * c:32 * c + 16, :],
                    ).then_inc(dma_out if c == 3 else dma_scrap, 16)

            for s in range(S):
                if s >= 2:
                    sync.wait_ge(pe_s, 5 * (s - 2) + 4)
                sync.dma_start(out=gxt[s % 2][:, :],
                               in_=bass.AP(gxB, s * GH, [[S * GH, 16], [1, GH]]),
                               ).then_inc(dma_gx[s % 2], 16)
                if s >= 2:
                    sync.wait_ge(hob_s, s - 1)
                    store_out(s - 2)
            for s in (S - 2, S - 1):
                sync.wait_ge(hob_s, s + 1)
                store_out(s)

        # ---------------- PE ----------------
        @block.tensor
        def _(tensor):
            tensor.wait_ge(dma_w, INIT_DMAS)
            # phase 1: bank g = my tokens x gate-chunk g (half*4 + gatetype)
            for T in range(TT):
                tensor.wait_ge(dma_x[T % 3], 16 * (T // 3 + 1))
                for g in range(8):
                    if T >= 1:
                        tensor.wait_ge(act_p1, 8 * (T - 1) + g + 1)
                    for j in range(8):
                        tensor.matmul(
                            p1b[g][:, :],
                            xtile[T % 3][:, j * 128:(j + 1) * 128],
                            wih_sb[:, (j * 8 + g) * 512:(j * 8 + g + 1) * 512],
                            start=(j == 0), stop=False,
                        )
                    mm = tensor.matmul(p1b[g][:, :], ones_sb[:, :],
                                       bias_sb[:, g * 512:(g + 1) * 512],
                                       start=False, stop=True)
                    mm.then_inc(pe_p1, 1)
            # phase 2
            tensor.wait_ge(act_p1, 8 * TT)
            # per step: 4 concurrent col-strip bands (tile_position derives
            # from out base partition 32c); band c = gate piece c [16, 512].
            # Issue round-robin across bands so all 4 strips stay fed.
            for s in range(S):
                par = s % 2
                tensor.wait_ge(dma_gx[par], 16 * (s // 2 + 1))
                if s >= 2 and not freerun:
                    tensor.wait_ge(act_s, 4 * (s - 2) + 3)   # pg[par] WAR
                if not no_ident:
                    for c in range(4):
                        tensor.matmul(pg[par][32 * c:32 * c + 16, :], ident_sb[:, :],
                                      gxt[par][:, c * 512:(c + 1) * 512],
                                      start=True, stop=False,
                                      tile_position=(0, 32 * c))
                if s >= 1 and not freerun:
                    tensor.wait_ge(dve_s, 5 * s)          # own hT pieces ready
                if seq_bands:
                    order1 = [(j, c) for c in range(4) for j in range(4)]
                    order2 = [(j, c) for c in range(4) for j in range(4, 8)]
                else:
                    order1 = [(j, c) for j in range(4) for c in range(4)]
                    order2 = [(j, c) for j in range(4, 8) for c in range(4)]
                for j, c in order1:
                    tensor.matmul(
                        pg[par][32 * c:32 * c + 16, :],
                        hbuf[par][:, 16 * j:16 * (j + 1)],
                        whh_sb[:, j * GH + c * 512:j * GH + (c + 1) * 512],
                        start=(no_ident and j == 0), stop=False,
                        tile_position=(0, 32 * c),
                    )
                if s >= 1 and rsem_wait:
                    tensor.wait_ge(rsem_all, 2 * s)       # partner pieces
                for j, c in order2:
                    mm = tensor.matmul(
                        pg[par][32 * c:32 * c + 16, :],
                        hbuf[par][:, 16 * j:16 * (j + 1)],
                        whh_sb[:, j * GH + c * 512:j * GH + (c + 1) * 512],
                        start=False, stop=(j == 7),
                        tile_position=(0, 32 * c),
                    )
                    if j == 7:
                        mm.then_inc(pe_s, 1)
                # hT transpose: piece p = band p [16,128] -> psh[:, 16p:16p+16]
                if not freerun:
                    tensor.wait_ge(dve_s, 5 * s + 4)
                # banded h -> hT pieces in one untiled matmul:
                # psh[q, 16p+b] = sum_k hsb[k, q] * E[k, 16p+b],  E[32p+b, 16p+b]=1
                tensor.matmul(psh[(s + 1) % 2][:, 0:64], hsb[par][:, :],
                              selmat_sb[:, :], start=True, stop=True
                              ).then_inc(pe_s, 1)
            tensor.wait_ge(act_s, 4 * S)

        # ---------------- ACT ----------------
        @block.scalar
        def _(scalar):
            scalar.wait_ge(dma_w, INIT_DMAS)
            for T in range(TT):
                if T >= 2:
                    scalar.wait_ge(dma_gxA[T % 2], 16 * (T // 2))
                for g in range(8):
                    scalar.wait_ge(pe_p1, 8 * T + g + 1)
                    scalar.activation(stag[T % 2][:, g * 512:(g + 1) * 512],
                                      p1b[g][:, :], AF.Identity).then_inc(act_p1, 1)
            # idmap
            for m in range(7):
                scalar.wait_ge(id_rsem[m], 2)
            scalar.dma_start(out=idmap[:, :], in_=idbuf[:, :]).then_inc(dma_id, 16)
            # phase 2 (banded): sig(i,f) [128,256], tanh g, sig o, tanh(c),
            # hob copy.  All ops span the 4 partition bands at once.
            for s in range(S):
                par = s % 2
                scalar.wait_ge(pe_s, 5 * s + 4)          # all 4 band stops
                scalar.activation(gates[:, 0:256], pg[par][:, 0:256], AF.Sigmoid
                                  ).then_inc(act_s, 1)
                scalar.activation(gates[:, 256:384], pg[par][:, 256:384], AF.Tanh
                                  ).then_inc(act_s, 1)
                scalar.activation(gates[:, 384:512], pg[par][:, 384:512], AF.Sigmoid
                                  ).then_inc(act_s, 1)
                scalar.wait_ge(dve_s, 5 * s + 3)
                scalar.activation(tanhc[par][:, :], cst[:, :], AF.Tanh
                                  ).then_inc(act_s, 1)
                scalar.wait_ge(dve_s, 5 * s + 4)
                if s >= 2:
                    scalar.wait_ge(dma_out, 16 * (s - 1))
                scalar.activation(hob[par][:, :], hsb[par][:, :], AF.Copy
                                  ).then_inc(hob_s, 1)
            scalar.wait_ge(dma_out, 16 * S)

        # ---------------- DVE ----------------
        @block.vector
        def _(vector):
            vector.wait_ge(dma_w, INIT_DMAS)
            for s in range(S):
                par = s % 2
                vector.wait_ge(act_s, 4 * s + 1)           # sig i,f
                if s >= 1:
                    vector.wait_ge(dve_s, 5 * (s - 1) + 3)
                vector.tensor_mul(t1[:, :], gates[:, 128:256], cst[:, :]
                                  ).then_inc(dve_s, 1)
                vector.wait_ge(act_s, 4 * s + 2)           # tanh g
                vector.tensor_mul(t2[:, :], gates[:, 0:128], gates[:, 256:384]
                                  ).then_inc(dve_s, 1)
                vector.tensor_add(cst[:, :], t1[:, :], t2[:, :]).then_inc(dve_s, 1)
                vector.wait_ge(act_s, 4 * s + 4)           # tanh c (+ sig o)
                if s >= 2:
                    vector.wait_ge(hob_s, s - 1)           # hob copy of s-2
                vector.tensor_mul(hsb[par][:, :], gates[:, 384:512], tanhc[par][:, :]
                                  ).then_inc(dve_s, 1)
                vector.wait_ge(pe_s, 5 * s + 5)            # transposes done
                vector.tensor_copy(hbuf[(s + 1) % 2][:, 0:64], psh[(s + 1) % 2][:, 0:64]
                                   ).then_inc(dve_s, 1)

        # ---------------- GPSIMD ----------------
        @block.gpsimd
        def _(gp):
            gp.load_library(library_config.remote_dma)

            def bcast(m, out_ap, in_ap, rs, ls):
                rdests = [None] * 8
                rdests[m] = (0, m)
                gp.remote_dma_broadcast(out_ap=out_ap, in_ap=in_ap,
                                        remote_sem=rs, local_sem=ls, rdests=rdests,
                                        ).then_inc(prep_s, 1)

            for m in range(1, 8):
                bcast(m, idbuf[:, m * 8:(m + 1) * 8], idbuf[:, 0:8],
                      id_rsem[m - 1], id_lsem[m - 1])
            gp.wait_ge(prep_s, 7)
            gp.wait_ge(dma_w, INIT_DMAS)
            gp.trigger_dma(count=7)
            # redistribute gx within pairs: [my toks, both halves] -> [pair toks, my half]
            gp.wait_ge(dma_gxA[0], 16 * ((TT + 1) // 2))
            if TT >= 2:
                gp.wait_ge(dma_gxA[1], 16 * (TT // 2))
            gp.collective_compute(
                "AllToAll",
                mybir.AluOpType.bypass,
                replica_groups=[list(range(NC))],
                ins=[gxA.ap().opt()],
                outs=[gxB.ap().opt()],
            ).then_inc(cc_sem, 1)
            gp.wait_ge(cc_sem, 1)
            if do_exchange:
                if S >= 2:
                    bcast(mpart, hbuf[1][:, 64:128], hbuf[1][:, 0:64],
                          rsem_all, lsem)
                for s in range(S):
                    gp.wait_ge(dve_s, 5 * (s + 1))
                    if s < S - 1:
                        gp.wait_ge(prep_s, 7 + s + 1)
                        gp.trigger_dma(count=1)
                    if s % LSEM_CHK == 0 and s >= LSEM_CHK:
                        gp.wait_ge(lsem, 16 * (s - LSEM_CHK + 1))
                    if s + 1 <= S - 2:
                        p2 = (s + 2) % 2
                        bcast(mpart, hbuf[p2][:, 64:128], hbuf[p2][:, 0:64],
                              rsem_all, lsem)
                gp.wait_ge(lsem, 16 * (S - 1))

    es.close()
    library_overlay.lower_extended_insts(nc)
    return nc


def _selmat():
    E = np.zeros((128, 64), np.float32)
    for p in range(4):
        for b in range(16):
            E[32 * p + b, 16 * p + b] = 1.0
    return E


def host_prepare(x, W_ih, W_hh, b_ih, b_hh, S, bf16=True):
    import ml_dtypes
    dtx = ml_dtypes.bfloat16 if bf16 else np.float32
    xs = np.ascontiguousarray(x[:, :S, :])
    # gate rows for half e, banded: piece c (128 h-dims) x (i, f, g, o) x 128
    # (torch gate blocks i=0, f=1, g=2, o=3)
    rows_of_half = [np.concatenate(
        [np.arange(g * H + e * HH + 128 * c, g * H + e * HH + 128 * (c + 1))
         for c in range(4) for g in range(4)])
        for e in range(2)]
    rows_all = np.concatenate(rows_of_half)           # half0 2048 then half1
    wih_full = W_ih[rows_all, :].T                    # [1024, 4096]
    wih_packed = np.ascontiguousarray(
        wih_full.reshape(8, 128, 8, 512).transpose(1, 0, 2, 3).reshape(128, 32768)
    ).astype(dtx)
    bias_full = (b_ih + b_hh).astype(np.float32)
    bias_perm = bias_full[rows_all].reshape(1, 4096)
    in_maps = []
    for r in range(NC):
        k, e = r >> 1, r & 1
        # x shard: 2 batches from EACH quarter (A2A equal-chunk layout),
        # b-major tokens ordered (quarter, u): tok = (2q+u)*S + t
        bidx = [16 * q + 2 * r + u for q in range(4) for u in range(2)]
        xT = np.ascontiguousarray(
            xs[bidx].transpose(2, 0, 1).reshape(I, 8 * S)).astype(dtx)
        whh_slice = W_hh[rows_of_half[e], :]          # [2048, 1024]
        colperm = np.concatenate([np.arange(e * HH, e * HH + HH),
                                  np.arange((1 - e) * HH, (1 - e) * HH + HH)])
        whh_perm = whh_slice[:, colperm].T            # [1024 slot-ordered, 2048]
        in_maps.append(dict(
            xT=xT,
            wih=wih_packed,
            whh=np.ascontiguousarray(
                whh_perm.reshape(8, 128, GH).transpose(1, 0, 2).reshape(128, 8 * GH)
            ).astype(dtx),
            bias=bias_perm,
            ones=np.ones((1, 128), np.float32),
            ident=np.eye(16).astype(dtx),
            identf=np.tile(np.eye(16, dtype=np.float32), (8, 1)),
            selmat=_selmat(),
            zeros=np.zeros((128, 512), np.float32),
            zerox=np.zeros((128, 128), dtx),
            rankvec=np.full((128, 8), float(r), np.float32),
        ))
    return in_maps


def host_gather(results, S):
    full = np.zeros((B, S, H), np.float32)
    for r in range(NC):
        k, e = r >> 1, r & 1
        o = np.asarray(results[r]["out"]).astype(np.float32).reshape(S, BQ, HH)
        full[16 * k:16 * (k + 1), :, e * HH:(e + 1) * HH] = o.transpose(1, 0, 2)
    return full


def read_obs(results):
    return [[int(round(float(np.asarray(results[r]["idmap"]).reshape(128, 64)[0, m * 8])))
             for m in range(8)] for r in range(NC)]


# ---------------- PJRT runner ----------------

import jax
from jax.sharding import Mesh, PartitionSpec
from jax.experimental.shard_map import shard_map
from concourse import bass2jax


class Runner:
    def __init__(self, nc, n_cores=8, pass_out_zeros=False):
        bass2jax.install_neuronx_cc_hook()
        self.nc = nc
        self.n_cores = n_cores
        partition_name = nc.partition_id_tensor.name if nc.partition_id_tensor else None
        in_names, out_names, out_avals = [], [], []
        for alloc in nc.m.functions[0].allocations:
            if not isinstance(alloc, mybir.MemoryLocationSet):
                continue
            name = alloc.memorylocations[0].name
            if alloc.kind == "ExternalInput":
                if name != partition_name:
                    in_names.append(name)
            elif alloc.kind == "ExternalOutput":
                out_names.append(name)
                out_avals.append(jax.core.ShapedArray(
                    tuple(alloc.tensor_shape), mybir.dt.np(alloc.dtype)))
        self.in_names, self.out_names, self.out_avals = in_names, out_names, out_avals
        self.pass_out_zeros = pass_out_zeros
        n_params = len(in_names)
        n_outs = len(out_names) if pass_out_zeros else 0
        in_names_all = (in_names + (out_names if pass_out_zeros else [])
                        + ([partition_name] if partition_name else []))

        def _body(*args):
            operands = list(args)
            if partition_name is not None:
                operands.append(bass2jax.partition_id_tensor())
            outs = bass2jax._bass_exec_p.bind(
                *operands, out_avals=tuple(out_avals), in_names=tuple(in_names_all),
                out_names=tuple(out_names), lowering_input_output_aliases=(),
                sim_require_finite=False, sim_require_nnan=False, nc=nc)
            return tuple(outs)

        devices = jax.devices()[:n_cores]
        self.mesh = Mesh(np.asarray(devices), ("core",))
        self.jitted = jax.jit(shard_map(
            _body, mesh=self.mesh,
            in_specs=(PartitionSpec("core"),) * (n_params + n_outs),
            out_specs=(PartitionSpec("core"),) * len(out_names), check_rep=False),
            keep_unused=True)
        self.dev_args = None

    def set_inputs(self, in_maps, only=None):
        n = self.n_cores
        if self.dev_args is None:
            n_extra = len(self.out_names) if self.pass_out_zeros else 0
            self.dev_args = [None] * (len(self.in_names) + n_extra)
            only = None
        for i, name in enumerate(self.in_names):
            if only is not None and name not in only:
                continue
            cat = np.concatenate([np.asarray(in_maps[c][name]) for c in range(n)], axis=0)
            self.dev_args[i] = jax.device_put(cat)
        if only is None and self.pass_out_zeros:
            for j, av in enumerate(self.out_avals):
                z = np.zeros((n * av.shape[0], *av.shape[1:]), av.dtype)
                self.dev_args[len(self.in_names) + j] = jax.device_put(z)

    def run(self):
        outs = self.jitted(*self.dev_args)
        jax.block_until_ready(outs)
        return outs

    def results(self, outs):
        n = self.n_cores
        res = []
        for c in range(n):
            d = {}
            for i, name in enumerate(self.out_names):
                a = np.asarray(outs[i])
                d[name] = a.reshape(n, *self.out_avals[i].shape)[c]
            res.append(d)
        return res


# ---------------- harness entry point ----------------

USE_BF16 = True
_CACHE = {}


def kernel(x, W_ih, W_hh, b_ih, b_hh):
    """Full-input distributed LSTM on 8 trn2 NeuronCores. Returns (B, S, H) f32."""
    x = np.ascontiguousarray(np.asarray(x, np.float32))
    W_ih = np.asarray(W_ih, np.float32)
    W_hh = np.asarray(W_hh, np.float32)
    b_ih = np.asarray(b_ih, np.float32)
    b_hh = np.asarray(b_hh, np.float32)
    S = x.shape[1]

    mpart = _CACHE.get("mpart", 1)
    if "runner" not in _CACHE:
        nc = build_nc(S, bf16=USE_BF16, mpart=mpart)
        _CACHE["runner"] = Runner(nc, NC)
        _CACHE["S"] = S
    assert _CACHE["S"] == S
    r = _CACHE["runner"]

    in_maps = host_prepare(x, W_ih, W_hh, b_ih, b_hh, S, bf16=USE_BF16)
    r.set_inputs(in_maps)
    res = r.results(r.run())
    obs = read_obs(res)
    if any(obs[c][mpart] != c ^ 1 for c in range(NC)):
        # physical neighbour slot for the pair-partner differs: find the slot
        # m* with obs[r][m*] == r^1 for all r, rebuild, re-run
        cand = [m for m in range(1, 8)
                if all(obs[c][m] == c ^ 1 for c in range(NC))]
        assert cand, f"no uniform partner slot in {obs}"
        _CACHE["mpart"] = cand[0]
        nc = build_nc(S, bf16=USE_BF16, mpart=cand[0])
        _CACHE["runner"] = Runner(nc, NC)
        r = _CACHE["runner"]
        r.set_inputs(in_maps)
        res = r.results(r.run())
    return host_gather(res, S)
